# revision 29
# baseline (speedup 1.0000x reference)
"""GCN block kernel for Trainium2 (8 NeuronCores, SPMD).

Computes: h = A @ (x @ W) + b; BatchNorm1d(train, biased var); LeakyReLU(0.2)
  x: [16384, 128] f32, A: [16384, 16384] f32, W: [128, 128], b/gamma/beta: [128]

Strategy (row-shard over output nodes, 8 cores x 2048 rows):
  - Reassociate as hT = W^T (x^T A_shard^T): the A^T stream is consumed by
    matmuls whose *weights* are x chunks loaded straight from DRAM, so there
    is no XW prologue on the PE before the memory-bound stream can start.
    z[d, j] = sum_n x[n, d] A[j0+j, n] accumulates over 128 k-chunks in a
    [128, 2048] PSUM tile; A^T is streamed in f16 (host-prearranged to the
    exact SBUF tile layout, so every DMA is long-contiguous), 1 MiB per
    dma_start alternating between the two HWDGE queues (sync/scalar), with
    params and the x pieces on the gpsimd SWDGE queue out of the way.
    The PE pipelines the accumulating 512-col matmuls at ~215-265ns
    (clock-phase dependent), so the stream is HBM-bound (~310-330 GB/s
    effective with all 8 cores streaming).
  - Tail: z -> f16 SBUF; 4 matmuls by W (f16) give hT strips in PSUM; BN
    stats come from a DVE free-axis reduce of z (mean via W^T (zsum/N)) and
    an ACT Square pass with accum (E[h^2]); the [128, 2] per-core stats
    cross 8 cores via AllGather + local DVE reduce (AllGather is the
    cheapest ncfw collective on this fabric, ~20-35us); LeakyReLU applied
    as one ACT Prelu per strip reading the f16 copy of hT with scale/bias
    folded; PE f16 transposes back to natural [n, f] layout; output DMA'd
    per 512-row strip on alternating queues.
  - Two early dummy AllGathers absorb the ~68us cold ncfw trigger cost; a
    dummy Sqrt preloads the ACT table; 8 dummy matmuls bridge the DMA
    ramp so the PE clock governor (HAM) starts the stream warm.
  - A post-compile pass strips redundant per-matmul LDWEIGHTS reloads.
Measured on 8 trn2 NeuronCores: ~260-310us (run-to-run variance from the
chip power-throttle phase and ncfw collective latency), rel err ~1.0e-3.
"""

import numpy as np

import concourse.bass as bass
import concourse.bacc as bacc
import concourse.mybir as mybir
import concourse.tile as tile
from concourse.bass_utils import run_bass_kernel_spmd

N = 16384
D = 128
NCORES = 8
R = N // NCORES          # 2048 rows per core
KCH = N // 128           # 128 k-chunks
# k-chunks per DMA group (1 MiB f16 per dma_start): fine granularity keeps
# the PE fed with minimal jitter; deep buffering rides out delivery bursts
RAMP_GROUPS = [2] * 8 + [4] * 28
assert sum(RAMP_GROUPS) == KCH
AT_BUFS = 6              # SBUF ring depth for A^T stream tiles (12 MiB)
EPS = 1e-5
NEG_SLOPE = 0.2

F32 = mybir.dt.float32
F16 = mybir.dt.float16


def build_program():
    nc = bacc.Bacc("TRN2", target_bir_lowering=False, debug=False,
                   num_devices=NCORES)

    at = nc.dram_tensor("at", [128, KCH * R], F16, kind="ExternalInput")
    xs = nc.dram_tensor("xs", [128, N], F16, kind="ExternalInput")
    # params packed into two blobs so the head costs two DMAs, not five:
    # p16 = [wh | ident], pf32 = [b | gamma | beta]
    p16 = nc.dram_tensor("p16", [D, 2 * D], F16, kind="ExternalInput")
    pf32 = nc.dram_tensor("pf32", [D, 3], F32, kind="ExternalInput")
    out = nc.dram_tensor("out", [R, D], F32, kind="ExternalOutput")

    with tile.TileContext(nc) as tc:
        with (
            tc.tile_pool(name="const", bufs=1) as cpool,
            tc.tile_pool(name="x", bufs=1) as xpool,
            tc.tile_pool(name="at", bufs=AT_BUFS) as atpool,
            tc.tile_pool(name="work", bufs=1) as wpool,
            tc.tile_pool(name="psum_z", bufs=1, space="PSUM") as pz,
            tc.tile_pool(name="psum_s", bufs=2, space="PSUM") as ps,
            tc.tile_pool(name="dram", bufs=1, space="DRAM") as dpool,
        ):
            # ---- head: params, x chunk-weights, collective + table warmup ----
            p16_sb = cpool.tile([D, 2 * D], F16)
            nc.gpsimd.dma_start(p16_sb[:], p16[:])
            wh_sb = p16_sb[:, 0:D]
            id_sb = p16_sb[:, D:2 * D]
            pf32_sb = cpool.tile([D, 3], F32)
            nc.gpsimd.dma_start(pf32_sb[:], pf32[:])
            b_sb = pf32_sb[:, 0:1]
            gam_sb = pf32_sb[:, 1:2]
            bet_sb = pf32_sb[:, 2:3]
            eps_sb = cpool.tile([D, 1], F32)
            nc.gpsimd.memset(eps_sb[:], EPS)

            # x (the stream matmuls' stationary operand), [p, chunk*128+d]
            x_sb = xpool.tile([D, N], F16)
            nc.sync.dma_start(x_sb[:, 0:4096], xs[:, 0:4096])

            # ACT: (R/N)*b for the mean path; Sqrt warms the ACT table slot
            b2048_sb = cpool.tile([D, 1], F32)
            nc.scalar.mul(b2048_sb[:], b_sb, float(R) / N)
            sqrt_warm = cpool.tile([D, 1], F32)
            nc.scalar.activation(sqrt_warm[:], eps_sb[:],
                                 mybir.ActivationFunctionType.Sqrt,
                                 bias=eps_sb[:])

            # warm the ncfw collective path twice so the tail stats
            # AllReduce runs on a hot control plane
            warm_sb = cpool.tile([D, 2], F32)
            nc.gpsimd.memset(warm_sb[:], 0.0)
            warm_in = dpool.tile([1, 2 * D], F32, name="warm_in")
            nc.gpsimd.dma_start(
                warm_in.rearrange("o (f t) -> (o f) t", t=2), warm_sb[:])
            for wi in range(2):
                warm_out = dpool.tile([1, 2 * D * NCORES], F32,
                                      addr_space="Shared",
                                      name=f"warm_out{wi}")
                nc.gpsimd.collective_compute(
                    "AllGather", mybir.AluOpType.bypass,
                    replica_groups=[list(range(NCORES))],
                    ins=[warm_in.opt()], outs=[warm_out.opt()])

            # ---- stream: z[d, j] = sum_k x[k, d] At[k, j] over 128 chunks ----
            # The PE pipelines the 512-col matmuls at ~215ns when warm, so
            # the stream is DMA-bound: three queues keep HBM at its ~358GB/s
            # ceiling. A few dummy matmuls bridge the DMA ramp-up so the
            # HAM clock never sees a >3.4us idle window at stream start.
            psum_z = pz.tile([D, R], F32)  # 4 PSUM banks
            pdum = ps.tile([D, 512], F32, tag="h", name="pdum")
            for w in range(8):
                nc.tensor.matmul(pdum[:], x_sb[:, bass.ts(w, D)],
                                 x_sb[:, bass.ts(w, 512)],
                                 start=True, stop=True)
            k0 = 0
            for g, cpd in enumerate(RAMP_GROUPS):
                at_t = atpool.tile([128, 4 * R], F16, tag="at")
                eng = nc.scalar if g % 2 == 0 else nc.sync
                eng.dma_start(at_t[:, 0:cpd * R],
                              at.ap()[:, k0 * R:(k0 + cpd) * R])
                if g in (1, 3, 5):
                    # x chunk-weight pieces ride the gpsimd queue, spread out
                    c = (g + 1) // 2
                    nc.gpsimd.dma_start(x_sb[:, c * 4096:(c + 1) * 4096],
                                        xs[:, c * 4096:(c + 1) * 4096])
                for a in range(cpd):
                    k = k0 + a
                    for s in range(4):
                        nc.tensor.matmul(
                            psum_z[:, bass.ts(s, 512)],
                            x_sb[:, bass.ts(k, D)],
                            at_t[:, a * R + 512 * s:a * R + 512 * (s + 1)],
                            start=(k == 0), stop=(k == KCH - 1),
                        )
                k0 += cpd

            # ---- tail: hT = W^T z + b, BN stats, AllReduce ----
            z16 = wpool.tile([D, R], F16)
            for s in range(4):
                nc.vector.tensor_copy(z16[:, bass.ts(s, 512)],
                                      psum_z[:, bass.ts(s, 512)])
            zsum = wpool.tile([D, 1], F32)
            nc.vector.reduce_sum(zsum[:], z16[:], axis=mybir.AxisListType.X)
            # pre-scale by 1/N (exact power of two) so the f16 cast can't
            # overflow; the mean path then needs no /N after the AllReduce
            zsumn = wpool.tile([D, 1], F32)
            nc.vector.tensor_scalar_mul(zsumn[:], zsum[:], 1.0 / N)
            zsum16 = wpool.tile([D, 1], F16)
            nc.vector.tensor_copy(zsum16[:], zsumn[:])

            h16 = wpool.tile([D, R], F16)
            hn16 = wpool.tile([D, R], F16)
            sq32 = wpool.tile([D, 512], F32)
            sums = wpool.tile([D, 4], F32)
            for s in range(4):
                psum_h = ps.tile([D, 512], F32, tag="h", name="psum_h")
                nc.tensor.matmul(psum_h[:], wh_sb, z16[:, bass.ts(s, 512)],
                                 start=True, stop=True)
                # sum of h^2 along the strip (h = psum_h + b); h^2 can top
                # 1e5, so the throwaway dst must be f32
                nc.scalar.activation(sq32[:], psum_h[:],
                                     mybir.ActivationFunctionType.Square,
                                     bias=b_sb, accum_out=sums[:, s:s + 1])
                nc.vector.tensor_copy(h16[:, bass.ts(s, 512)], psum_h[:])
            # mean path: this core's mean contribution = W^T (zsum/N) + (R/N)*b
            psum_m = ps.tile([D, 4], F32, tag="h", name="psum_m")
            nc.tensor.matmul(psum_m[:, 0:1], wh_sb, zsum16[:],
                             start=True, stop=True)

            cc_sb = wpool.tile([D, 2], F32)
            nc.vector.tensor_add(cc_sb[:, 0:1], psum_m[:, 0:1], b2048_sb[:])
            nc.vector.reduce_sum(cc_sb[:, 1:2], sums[:, 0:4],
                                 axis=mybir.AxisListType.X)

            # AllGather (cheaper than AllReduce on this fabric) + local
            # reduce over the 8 per-core stats vectors
            cc_in = dpool.tile([1, 2 * D], F32, name="cc_in")
            cc_out = dpool.tile([1, 2 * D * NCORES], F32, addr_space="Shared",
                                name="cc_out")
            nc.gpsimd.dma_start(
                cc_in.rearrange("o (f t) -> (o f) t", t=2), cc_sb[:])
            nc.gpsimd.collective_compute(
                "AllGather", mybir.AluOpType.bypass,
                replica_groups=[list(range(NCORES))],
                ins=[cc_in.opt()], outs=[cc_out.opt()])
            stats8 = wpool.tile([D, 2, NCORES], F32)
            nc.sync.dma_start(
                stats8[:],
                cc_out.rearrange("o (c f t) -> (o f) t c",
                                      c=NCORES, t=2))
            stats_g = wpool.tile([D, 2], F32)
            nc.vector.reduce_sum(stats_g[:], stats8[:],
                                 axis=mybir.AxisListType.X)

            # ---- per-feature scale/shift (stats_g[:,0] is already the mean) ----
            mean = stats_g[:, 0:1]
            ex2 = wpool.tile([D, 1], F32)
            nc.scalar.mul(ex2[:], stats_g[:, 1:2], 1.0 / N)
            msq = wpool.tile([D, 1], F32)
            nc.vector.tensor_mul(msq[:], mean, mean)
            var = wpool.tile([D, 1], F32)
            nc.vector.tensor_sub(var[:], ex2[:], msq[:])
            std = wpool.tile([D, 1], F32)
            nc.scalar.activation(std[:], var[:],
                                 mybir.ActivationFunctionType.Sqrt,
                                 bias=eps_sb[:])
            istd = wpool.tile([D, 1], F32)
            nc.vector.reciprocal(istd[:], std[:])
            scl = wpool.tile([D, 1], F32)
            nc.vector.tensor_mul(scl[:], gam_sb, istd[:])
            tmp = wpool.tile([D, 1], F32)
            nc.vector.tensor_mul(tmp[:], mean, scl[:])
            shf = wpool.tile([D, 1], F32)
            nc.vector.tensor_sub(shf[:], bet_sb, tmp[:])
            # h16 holds h - b, so fold the bias back in: scl*b + shf
            sclb = wpool.tile([D, 1], F32)
            nc.vector.tensor_mul(sclb[:], scl[:], b_sb)
            biasp = wpool.tile([D, 1], F32)
            nc.vector.tensor_add(biasp[:], shf[:], sclb[:])

            # ---- y = LeakyReLU(scl*h + biasp); transpose to [n, f]; store ----
            for s in range(4):
                nc.scalar.activation(hn16[:, bass.ts(s, 512)],
                                     h16[:, bass.ts(s, 512)],
                                     mybir.ActivationFunctionType.Prelu,
                                     bias=biasp[:], scale=scl[:],
                                     alpha=NEG_SLOPE)
            out_sb = wpool.tile([128, R], F32)
            out_ap = out.ap().rearrange("(t p) f -> p t f", p=128)
            for t in range(R // 128):
                ptr = ps.tile([D, D], F16, tag="ptr", name="ptr")
                nc.tensor.transpose(ptr[:], hn16[:, bass.ts(t, D)], id_sb)
                if t % 2 == 0:
                    nc.scalar.copy(out_sb[:, bass.ts(t, D)], ptr[:])
                else:
                    nc.vector.tensor_copy(out_sb[:, bass.ts(t, D)], ptr[:])
                if t % 4 == 3:
                    s = t // 4
                    eng = nc.sync if s % 2 == 0 else nc.scalar
                    eng.dma_start(
                        out_ap[:, 4 * s:4 * (s + 1), :],
                        out_sb[:, bass.ts(s, 512)].rearrange(
                            "p (t f) -> p t f", f=D))

    nc.compile()
    _dedupe_ldweights(nc.m)
    return nc


def _ldw_sig(ins):
    return (repr(ins.ins[0]), repr(ins.perf_mode), repr(ins.is_transpose),
            repr(ins.tile_position), repr(ins.tile_size))


def _dedupe_ldweights(m):
    """Drop back-to-back InstLdweights that reload identical weights.

    bacc emits one LDWEIGHTS per matmul; the 4 same-weight matmuls per
    k-chunk then reload the PE array 3 extra times, serializing the MM
    stream. Dupes carry no sync_info, so removal is safe; any transpose
    or differing load resets the tracked signature.
    """
    removed = 0
    for f in m.functions:
        for bb in f.blocks:
            last_sig = None
            keep = []
            for ins in bb.instructions:
                tn = type(ins).__name__
                if tn == "InstLdweights":
                    si = ins.sync_info
                    clean = si is None or (not si.on_wait and not si.on_update)
                    sig = _ldw_sig(ins)
                    if clean and sig == last_sig:
                        removed += 1
                        continue
                    last_sig = sig
                elif tn == "InstMatmult" and ins.is_transpose:
                    last_sig = None
                keep.append(ins)
            bb.instructions[:] = keep
    return removed


_CACHED = {}


def _get_program():
    if "nc" not in _CACHED:
        _CACHED["nc"] = build_program()
    return _CACHED["nc"]


def _make_in_maps(x, A, W, b, gamma, beta):
    x = np.asarray(x, dtype=np.float32)
    A = np.asarray(A, dtype=np.float32)
    W = np.asarray(W, dtype=np.float32)
    b = np.asarray(b, dtype=np.float32).reshape(D, 1)
    gamma = np.asarray(gamma, dtype=np.float32).reshape(D, 1)
    beta = np.asarray(beta, dtype=np.float32).reshape(D, 1)

    # xs[p, c*128 + d] = x[c*128 + p, d]
    xs = np.ascontiguousarray(
        x.reshape(KCH, 128, D).transpose(1, 0, 2)).astype(np.float16)
    xs = xs.reshape(128, N)
    p16 = np.concatenate([W.astype(np.float16),
                          np.eye(D, dtype=np.float16)], axis=1)
    pf32 = np.concatenate([b, gamma, beta], axis=1).astype(np.float32)
    common = {"xs": xs, "p16": np.ascontiguousarray(p16),
              "pf32": np.ascontiguousarray(pf32)}
    in_maps = []
    for j in range(NCORES):
        # atp[p, c*R + jj] = A[j*R + jj, c*128 + p]
        shard = A[j * R:(j + 1) * R, :]
        atp = shard.reshape(R, KCH, 128).transpose(2, 1, 0).astype(np.float16)
        m = dict(common)
        m["at"] = atp.reshape(128, KCH * R)
        in_maps.append(m)
    return in_maps


def run(x, A, W, b, gamma, beta, trace=False):
    nc = _get_program()
    in_maps = _make_in_maps(x, A, W, b, gamma, beta)
    res = run_bass_kernel_spmd(nc, in_maps, core_ids=list(range(NCORES)),
                               trace=trace)
    shards = [res.results[j]["out"] for j in range(NCORES)]
    full = np.concatenate(shards, axis=0)
    return full, res


def kernel(x, A, W, b, gamma, beta):
    full, _ = run(x, A, W, b, gamma, beta, trace=False)
    return full
